# revision 30
# baseline (speedup 1.0000x reference)
"""Trainium2 Bass kernel for nn_CompleteVQVAETextModel (VQ-VAE text encoder).

Pipeline per batch element (data-parallel over 8 NeuronCores, 4 batches each):
  conv1d(256->256, k3 s2 p1) + BN + LeakyReLU      (PE matmuls + ScalarE Lrelu)
  conv1d(256->512, k3 s2 p1) + BN + LeakyReLU
  conv1d(512->128, k3 s1 p1) + bias
  VQ: dist[n,e] = ||z_n||^2 + ||e||^2 - 2 z.e  over codebook [4096,128]
      codes = argmin_e dist (fp32, lowest-index tie-break), min_dist = min_e dist

Distance reduction: PE computes z.e into PSUM; ScalarE adds -z2/2 in place;
DVE tensor_tensor_reduce computes (psum - e2/2)*(-2) = dist (written to SBUF)
fused with a running min per 1024-wide chunk; argmin recovered with a
max_index (first-match) scan over the SBUF dist row.
"""

import numpy as np

import concourse.bass as bass
import concourse.mybir as mybir
import concourse.tile as tile
from concourse import bacc
from concourse.masks import make_identity

F32 = mybir.dt.float32
I32 = mybir.dt.int32
U32 = mybir.dt.uint32
ALU = mybir.AluOpType
ACTF = mybir.ActivationFunctionType

BN_EPS = 1e-5
LRELU_SLOPE = 0.01
N_CORES = 8
B = 32
C_IN, L = 256, 4096
K_CODES, D = 4096, 128
L1, L2 = 2048, 1024  # conv1/conv2 output lengths
BIG = 3.0e38


def build_nc(b_loc=B // N_CORES, compile=True):
    nc = bacc.Bacc("TRN2", debug=False)

    def din(name, shape):
        return nc.dram_tensor(name, shape, F32, kind="ExternalInput").ap()

    x = din("x", [b_loc, C_IN, L])
    w1 = din("conv1_w", [256, 256, 3])
    b1 = din("conv1_b", [256])
    g1, bb1, m1, v1 = (din(f"bn1_{s}", [256]) for s in "gbmv")
    w2 = din("conv2_w", [512, 256, 3])
    b2 = din("conv2_b", [512])
    g2, bb2, m2, v2 = (din(f"bn2_{s}", [512]) for s in "gbmv")
    w3 = din("conv3_w", [D, 512, 3])
    b3 = din("conv3_b", [D])
    cb = din("codebook", [K_CODES, D])

    codes = nc.dram_tensor("codes", [b_loc, L2], I32, kind="ExternalOutput").ap()
    md = nc.dram_tensor("min_dist", [b_loc, L2], F32, kind="ExternalOutput").ap()

    with tile.TileContext(nc) as tc:
        _body(tc, b_loc, x, w1, b1, g1, bb1, m1, v1, w2, b2, g2, bb2, m2, v2,
              w3, b3, cb, codes, md)
    if compile:
        nc.compile()
    return nc


def _bn_prep(nc, pool, setup, g, bb, m, v, b, n_ch, name, eps_col):
    """Fold conv bias + eval-mode batchnorm into per-channel scale/bias columns.

    Returns (scale, bias) SBUF tiles [128, n_ch//128]:
      scale[o] = g/sqrt(v+eps); bias[o] = (b - m)*scale + bb
    """
    nch = n_ch // 128
    sc = pool.tile([128, nch], F32, name=f"{name}_sc", tag=f"{name}_sc")
    bi = pool.tile([128, nch], F32, name=f"{name}_bi", tag=f"{name}_bi")
    gv, bv, mv, vv = (t.rearrange("(c p) -> p c", p=128) for t in (g, bb, m, v))
    bcv = b.rearrange("(c p) -> p c", p=128)
    tg = setup.tile([128, nch], F32, name=f"{name}_tg", tag=f"{name}_tg")
    tb = setup.tile([128, nch], F32, name=f"{name}_tb", tag=f"{name}_tb")
    tm = setup.tile([128, nch], F32, name=f"{name}_tm", tag=f"{name}_tm")
    tv = setup.tile([128, nch], F32, name=f"{name}_tv", tag=f"{name}_tv")
    nc.sync.dma_start(tg, gv)
    nc.sync.dma_start(tb, bv)
    nc.sync.dma_start(tm, mv)
    nc.sync.dma_start(tv, vv)
    # sc = g / sqrt(v + eps)
    nc.scalar.activation(sc, tv, ACTF.Sqrt, bias=eps_col, scale=1.0)
    nc.vector.reciprocal(sc, sc)
    nc.vector.tensor_tensor(sc, sc, tg, op=ALU.mult)
    # bi = (conv_b - m) * sc + bn_b
    bcol = setup.tile([128, nch], F32, name=f"{name}_bcol", tag=f"{name}_bcol")
    nc.sync.dma_start(bcol, bcv)
    nc.vector.tensor_tensor(bi, bcol, tm, op=ALU.subtract)
    nc.vector.tensor_tensor(bi, bi, sc, op=ALU.mult)
    nc.vector.tensor_tensor(bi, bi, tb, op=ALU.add)
    return sc, bi


def _body(tc, b_loc, x, w1, b1, g1, bb1, m1, v1, w2, b2, g2, bb2, m2, v2,
          w3, b3, cb, codes, md):
    nc = tc.nc
    P = 128

    const = tc.alloc_tile_pool(name="const", bufs=1, space="SBUF")

    # ---------------- setup: constants, weight transposes, codebook ----------
    with tc.tile_pool(name="setup", bufs=1, space="SBUF") as setup, \
         tc.tile_pool(name="setup_ps", bufs=2, space="PSUM") as setup_ps:

        ident = const.tile([P, P], F32, name="ident", tag="ident")
        make_identity(nc, ident)
        ones_col = const.tile([P, 1], F32, name="ones_col", tag="ones_col")
        nc.vector.memset(ones_col, 1.0)
        ones_row1 = const.tile([1, P], F32, name="ones_row1", tag="ones_row1")
        nc.vector.memset(ones_row1, 1.0)
        zeros8 = const.tile([P, 8], F32, name="zeros8", tag="zeros8")
        nc.vector.memset(zeros8, 0.0)
        zero_col = const.tile([P, 1], F32, name="zero_col", tag="zero_col")
        nc.vector.memset(zero_col, 0.0)
        eps_col = const.tile([P, 1], F32, name="eps_col", tag="eps_col")
        nc.vector.memset(eps_col, BN_EPS)

        s1, bi1 = _bn_prep(nc, const, setup, g1, bb1, m1, v1, b1, 256, "bn1",
                           eps_col)
        s2, bi2 = _bn_prep(nc, const, setup, g2, bb2, m2, v2, b2, 512, "bn2",
                           eps_col)
        b3col = const.tile([P, 1], F32, name="b3col", tag="b3col")
        nc.sync.dma_start(b3col, b3.rearrange("(c p) -> p c", p=128))

        # conv weights: DRAM [O, I, 3] -> lhsT tiles [128 i, 128 o] per (oc, ic, k)
        # via PE transpose of strided slices of the raw [128 o, I*3] rows.
        def load_wT(w, n_oc, n_ic, name):
            wT = const.tile([P, n_oc * n_ic * 3 * P], F32, name=f"{name}T",
                            tag=f"{name}T")
            idx = 0
            for o in range(n_oc):
                raw = setup.tile([P, n_ic * 384], F32, name=f"{name}_raw",
                                 tag=f"{name}_raw", bufs=2)
                nc.sync.dma_start(
                    raw, w[o * 128:(o + 1) * 128].rearrange("o i k -> o (i k)"))
                for i in range(n_ic):
                    for k in range(3):
                        ps = setup_ps.tile([P, P], F32, name=f"{name}_ps",
                                           tag="wps")
                        src = raw[:, i * 384 + k:(i + 1) * 384:3]
                        nc.tensor.transpose(ps, src, ident)
                        nc.scalar.copy(wT[:, idx * P:(idx + 1) * P], ps)
                        idx += 1
            return wT

        w1T = load_wT(w1, 2, 2, "w1")   # idx = (o*2 + i)*3 + k
        w2T = load_wT(w2, 4, 2, "w2")
        w3T = load_wT(w3, 1, 4, "w3")

        # codebook: cbT [128 d, 4096 e]; e2row [1, 4096] = codebook sq-norms
        cbT = const.tile([P, K_CODES], F32, name="cbT", tag="cbT")
        for c in range(32):
            craw = setup.tile([P, P], F32, name="craw", tag="craw", bufs=4)
            nc.sync.dma_start(craw, cb[c * 128:(c + 1) * 128, :])
            ps = setup_ps.tile([P, P], F32, name="cb_ps", tag="wps")
            nc.tensor.transpose(ps, craw, ident)
            nc.scalar.copy(cbT[:, c * 128:(c + 1) * 128], ps)
        sqT = setup.tile([P, K_CODES], F32, name="sqT", tag="sqT")
        nc.scalar.activation(sqT, cbT, ACTF.Square, bias=zero_col, scale=1.0)
        e2row = const.tile([1, K_CODES], F32, name="e2row", tag="e2row")
        for j in range(8):
            ps1 = setup_ps.tile([1, 512], F32, name="e2r_ps", tag="e2r_ps")
            nc.tensor.matmul(ps1, ones_col, sqT[:, j * 512:(j + 1) * 512],
                             start=True, stop=True)
            nc.scalar.copy(e2row[:, j * 512:(j + 1) * 512], ps1)
        # conv3 bias scaled by -2 (z path carries a -2 factor)
        b3neg2 = const.tile([P, 1], F32, name="b3neg2", tag="b3neg2")
        nc.vector.tensor_scalar_mul(b3neg2, b3col, -2.0)

    # ---------------- per-batch main loop ------------------------------------
    data = tc.alloc_tile_pool(name="data", bufs=1, space="SBUF")
    small = tc.alloc_tile_pool(name="small", bufs=2, space="SBUF")
    conv_ps = tc.alloc_tile_pool(name="conv_ps", bufs=2, space="PSUM")
    vq_ps = tc.alloc_tile_pool(name="vq_ps", bufs=2, space="PSUM")
    z2_ps = tc.alloc_tile_pool(name="z2_ps", bufs=1, space="PSUM")

    XW = L + 2       # padded x row width
    H1W = L1 + 2
    H2W = L2 + 2
    for b in range(b_loc):
        xp = [data.tile([P, XW], F32, name=f"xp{c}", tag=f"xp{c}", bufs=2)
              for c in range(2)]
        for c in range(2):
            nc.vector.memset(xp[c][:, 0:1], 0.0)
            nc.vector.memset(xp[c][:, XW - 1:XW], 0.0)
            nc.sync.dma_start(xp[c][:, 1:1 + L], x[b, c * 128:(c + 1) * 128, :])

        # conv1: out [256, 2048] = sum_{k,ic} w1T[ic,:,k].T @ xp[ic, 2l'+k]
        h1 = [data.tile([P, H1W], F32, name=f"h1_{c}", tag=f"h1_{c}")
              for c in range(2)]
        for c in range(2):
            nc.vector.memset(h1[c][:, 0:1], 0.0)
            nc.vector.memset(h1[c][:, H1W - 1:H1W], 0.0)
        for o in range(2):
            for n in range(4):
                ps = conv_ps.tile([P, 512], F32, name="c1ps", tag="cps")
                first = True
                for i in range(2):
                    for k in range(3):
                        widx = (o * 2 + i) * 3 + k
                        rhs = xp[i][:, n * 1024 + k:n * 1024 + k + 1024:2]
                        nc.tensor.matmul(ps, w1T[:, widx * P:(widx + 1) * P],
                                         rhs, start=first, stop=(i == 1 and k == 2))
                        first = False
                nc.scalar.activation(
                    h1[o][:, 1 + n * 512:1 + (n + 1) * 512],
                    ps, ACTF.Lrelu, bias=bi1[:, o:o + 1], scale=s1[:, o:o + 1],
                    alpha=LRELU_SLOPE)

        # conv2: out [512, 1024]
        h2 = [data.tile([P, H2W], F32, name=f"h2_{c}", tag=f"h2_{c}")
              for c in range(4)]
        for c in range(4):
            nc.vector.memset(h2[c][:, 0:1], 0.0)
            nc.vector.memset(h2[c][:, H2W - 1:H2W], 0.0)
        for o in range(4):
            for n in range(2):
                ps = conv_ps.tile([P, 512], F32, name="c2ps", tag="cps")
                first = True
                for i in range(2):
                    for k in range(3):
                        widx = (o * 2 + i) * 3 + k
                        rhs = h1[i][:, n * 1024 + k:n * 1024 + k + 1024:2]
                        nc.tensor.matmul(ps, w2T[:, widx * P:(widx + 1) * P],
                                         rhs, start=first, stop=(i == 1 and k == 2))
                        first = False
                nc.scalar.activation(
                    h2[o][:, 1 + n * 512:1 + (n + 1) * 512],
                    ps, ACTF.Lrelu, bias=bi2[:, o:o + 1], scale=s2[:, o:o + 1],
                    alpha=LRELU_SLOPE)

        # conv3: zT [128 d, 1024 tok]
        zT = data.tile([P, L2], F32, name="zT", tag="zT")
        for n in range(2):
            ps = conv_ps.tile([P, 512], F32, name="c3ps", tag="cps")
            first = True
            for i in range(4):
                for k in range(3):
                    widx = i * 3 + k
                    rhs = h2[i][:, n * 512 + k:n * 512 + k + 512]
                    nc.tensor.matmul(ps, w3T[:, widx * P:(widx + 1) * P],
                                     rhs, start=first, stop=(i == 3 and k == 2))
                    first = False
            # zT2 = -2*z  (fold the -2 of the distance into the z path)
            nc.scalar.activation(zT[:, n * 512:(n + 1) * 512], ps, ACTF.Identity,
                                 bias=b3neg2[:, 0:1], scale=-2.0)

        # z2 = sum_d z^2 per token (zT holds -2z, so sq = 4 z^2, scale 0.25)
        sq = data.tile([P, L2], F32, name="sq", tag="vpre", bufs=2)
        nc.scalar.activation(sq, zT, ACTF.Square, bias=zero_col, scale=1.0)
        z2r = small.tile([1, L2], F32, name="z2r", tag="z2r", bufs=1)
        for h in range(2):
            z2p = z2_ps.tile([1, 512], F32, name="z2p", tag="z2p", bufs=1)
            nc.tensor.matmul(z2p, ones_col, sq[:, h * 512:(h + 1) * 512],
                             start=True, stop=True)
            nc.scalar.activation(z2r[:, h * 512:(h + 1) * 512], z2p,
                                 ACTF.Copy, bias=0.0, scale=0.25)
        # transpose z2r row into a [128 tok, 8 ttile] column tile via K=1 matmuls
        nzps = z2_ps.tile([P, 8], F32, name="nzps", tag="nzps")
        for t in range(8):
            nc.tensor.matmul(nzps[:, t:t + 1], z2r[:, t * 128:(t + 1) * 128],
                             ones_row1[0:1, 0:1], start=True, stop=True)
        pz2 = small.tile([P, 8], F32, name="pz2", tag="pz2")
        nc.scalar.activation(pz2, nzps, ACTF.Copy, bias=0.0, scale=1.0)

        # VQ per 128-token tile
        for t in range(8):
            v = data.tile([P, K_CODES], F32, name="v", tag="v", bufs=1)
            cmin = small.tile([P, 4], F32, name="cmin", tag="cmin")
            md_col = small.tile([P, 1], F32, name="md_col", tag="md_col")
            for cp in range(4):
                ps = vq_ps.tile([P, 1024], F32, name="vqps", tag="vqps")
                for h in range(2):
                    sl = slice(cp * 1024 + h * 512, cp * 1024 + (h + 1) * 512)
                    # psum = e2 - 2 z.e (e2 seeded by a K=1 ones x e2row matmul)
                    nc.tensor.matmul(ps[:, h * 512:(h + 1) * 512],
                                     ones_row1, e2row[:, sl],
                                     start=True, stop=False)
                    nc.tensor.matmul(ps[:, h * 512:(h + 1) * 512],
                                     zT[:, t * 128:(t + 1) * 128], cbT[:, sl],
                                     start=False, stop=True)
                # v = psum + z2 = full squared distance (ScalarE evict)
                nc.scalar.activation(v[:, cp * 1024:(cp + 1) * 1024], ps,
                                     ACTF.Identity, bias=pz2[:, t:t + 1],
                                     scale=1.0)
                nc.vector.tensor_reduce(cmin[:, cp:cp + 1],
                                        v[:, cp * 1024:(cp + 1) * 1024],
                                        axis=mybir.AxisListType.X, op=ALU.min)
            nc.vector.tensor_reduce(md_col, cmin,
                                    axis=mybir.AxisListType.X, op=ALU.min)
            m8 = small.tile([P, 8], F32, name="m8", tag="m8")
            nc.vector.tensor_scalar(m8, zeros8, md_col, None, op0=ALU.add)
            idx8 = small.tile([P, 8], U32, name="idx8", tag="idx8")
            nc.vector.max_index(idx8, m8, v)
            codes_col = small.tile([P, 1], I32, name="codes_col",
                                   tag="codes_col")
            nc.vector.tensor_copy(codes_col, idx8[:, 0:1].bitcast(I32))
            nc.sync.dma_start(
                codes[b, t * 128:(t + 1) * 128].rearrange("p -> p ()"),
                codes_col)
            nc.sync.dma_start(
                md[b, t * 128:(t + 1) * 128].rearrange("p -> p ()"), md_col)

    for pool in (z2_ps, vq_ps, conv_ps, small, data, const):
        pool.release()


# ---------------- host wrapper ----------------------------------------------
_CACHED = {}


def kernel(**inputs):
    from concourse import bass_utils

    b_loc = B // N_CORES
    key = "nc"
    if key not in _CACHED:
        _CACHED[key] = build_nc(b_loc)
    nc = _CACHED[key]

    def f32(a):
        return np.ascontiguousarray(np.asarray(a, dtype=np.float32))

    shared = {k: f32(v) for k, v in inputs.items() if k != "x"}
    x = f32(inputs["x"])
    in_maps = []
    for c in range(N_CORES):
        m = dict(shared)
        m["x"] = np.ascontiguousarray(x[c * b_loc:(c + 1) * b_loc])
        in_maps.append(m)

    res = bass_utils.run_bass_kernel_spmd(nc, in_maps, core_ids=list(range(N_CORES)))
    codes = np.concatenate([r["codes"] for r in res.results], axis=0)
    min_dist = np.concatenate([r["min_dist"] for r in res.results], axis=0)
    return codes.astype(np.int32), min_dist.astype(np.float32)
